# revision 1
# baseline (speedup 1.0000x reference)
"""Griffin-Lim phase reconstruction on Trainium2 (Bass/Tile).

Key observations exploited here:
  * The reference returns only wav[:, 15:1015] -- the first 1000 samples of a
    32224-sample overlap-add waveform.  Influence propagates at most +-7
    frames per Griffin-Lim iteration and is strongly attenuated by the Hann
    window tails, so only the first TC=80 (of 1000) STFT frames can affect
    the output (validated numerically: rel err ~5e-6 = the fp32 noise floor;
    the cliff is below 48 frames).
  * The phase never needs to be materialized: carrying (mag*cos, mag*sin) and
    renormalizing with Re/|z|, Im/|z| reproduces angle()+exp() exactly, so no
    atan2/sin/cos in the loop (and |z| errors do not accumulate because the
    next iteration rescales by the fixed magnitude anyway).
  * irfft/rfft of 256 points are dense fp32 matmuls.  Per iteration only 8
    K=128 matmuls run:
      - ISTFT: 4 matmuls produce the windowed frames in (sample, frame)
        layout (two 128-sample halves x {cos,sin} basis accumulated in PSUM);
        the overlap-add collapses the 8 hop-shifted 32-partition groups with
        in-place DVE adds (partition-base-shifted PSUM operands).
      - STFT: the frame gather is 8 hop-shifted copies of the waveform tile
        into two 128-partition operands (DVE/ScalarE partition-shifted
        copies), then 4 matmuls against the windowed DFT basis.
  * ScalarE executes only Sqrt (single LUT, no table switches); everything
    else elementwise runs on VectorE.

Layouts (per core; core c handles batch element c%4, cores 4-7 duplicate):
  SA[128, 7+TC+7]  spec chunk A: rows f=0..127 of mag*cos(theta), zero-padded
  SB[...]          chunk B: row0 = Nyquist mag*cos, rows 1..127 = mag*sin
  P1/P2[128, TC]   frames: P1[n,m]/P2[128+n,m] = windowed sample n of frame m
  Wn[32, TC]       waveform as wav[32*m + i] at (partition i, column m)
  T2R/T2I[128, TS] STFT output: T2R = Re[f=0..127]; T2I row0 = Re[Nyquist],
                   rows 1..127 = Im[f=1..127]   (Im at f=0 and Nyquist == 0)
"""

import numpy as np
from contextlib import ExitStack

import concourse.bass as bass
import concourse.tile as tile
from concourse import bacc, mybir
from concourse import bass_utils

F32 = mybir.dt.float32
AF = mybir.ActivationFunctionType
OP = mybir.AluOpType

TC = 80           # cropped frame count (of 1000)
TS = TC - 7       # stft / phase-update frame count
PAD = 7
N_ITER = 32
N_FFT = 256
NF = 129
HOP = 32
N_CORES = 8
B = 4


def _consts():
    n = np.arange(N_FFT, dtype=np.float64)
    win = 0.5 - 0.5 * np.cos(2.0 * np.pi * n / N_FFT)
    k = np.arange(128, dtype=np.float64)[:, None]
    ang = 2.0 * np.pi * k * n[None, :] / N_FFT
    ck = np.where(k == 0, 1.0, 2.0) / N_FFT
    a_r = (ck * np.cos(ang) * win[None, :]).astype(np.float32)       # (128,256)
    a_i = (-2.0 / N_FFT * np.sin(ang) * win[None, :]).astype(np.float32)
    a_i[0] = (np.cos(np.pi * n) / N_FFT * win).astype(np.float32)    # Nyquist row

    f = np.arange(128, dtype=np.float64)[None, :]
    ang2 = 2.0 * np.pi * f * n[:, None] / N_FFT                      # (256,128)
    bc = (win[:, None] * np.cos(ang2)).astype(np.float32)
    bi = (-win[:, None] * np.sin(ang2)).astype(np.float32)
    bi[:, 0] = (win * np.cos(np.pi * n)).astype(np.float32)

    L = TC * HOP
    wsq = np.zeros((TC + 8) * HOP + N_FFT, dtype=np.float64)
    w2 = win ** 2
    for t in range(TC + 8):
        s = t * HOP
        wsq[s:s + N_FFT] += w2
    wsq = np.maximum(wsq[:L], 1e-8)
    invwsq = (1.0 / wsq).astype(np.float32).reshape(TC, HOP).T.copy()  # (32, TC)
    return a_r, a_i, bc.copy(), bi.copy(), invwsq


def _emit(tc_ctx, aps, rep=1):
    tc = tc_ctx
    nc = tc.nc
    with ExitStack() as ctx:
        consts = ctx.enter_context(tc.tile_pool(name="consts", bufs=1))
        state = ctx.enter_context(tc.tile_pool(name="state", bufs=1))
        work = ctx.enter_context(tc.tile_pool(name="work", bufs=3))
        psum = ctx.enter_context(tc.tile_pool(name="psum", bufs=2, space="PSUM"))

        a_r = consts.tile([128, 256], F32)
        a_i = consts.tile([128, 256], F32)
        bca = consts.tile([128, 128], F32)
        bcb = consts.tile([128, 128], F32)
        bia = consts.tile([128, 128], F32)
        bib = consts.tile([128, 128], F32)
        invw = consts.tile([32, TC], F32)
        maga = consts.tile([128, TS], F32)
        magn = consts.tile([1, TS], F32)
        sa = state.tile([128, TC + 2 * PAD], F32)
        sb = state.tile([128, TC + 2 * PAD], F32)
        epsb = consts.tile([128, 1], F32)
        nc.vector.memset(epsb, 1e-6)

        for t, name in [(a_r, "a_r"), (a_i, "a_i"), (bca, "bca"), (bcb, "bcb"),
                        (bia, "bia"), (bib, "bib"),
                        (invw, "invw"), (maga, "maga"), (magn, "magn")]:
            nc.sync.dma_start(out=t, in_=aps[name])

        if rep > 1:
            from concourse.engine_type import EngineType
            loop = tc.For_i(0, rep, 1, hint_engines=(
                EngineType.PE, EngineType.DVE, EngineType.Activation,
                EngineType.SP))
        else:
            loop = None
        if loop is not None:
            loop.__enter__()
        nc.sync.dma_start(out=sa, in_=aps["sa0"])
        nc.sync.dma_start(out=sb, in_=aps["sb0"])

        for it in range(N_ITER):
            last = it == N_ITER - 1
            # ---- ISTFT: frames in (n, m) layout via 4 K=128 matmuls ----
            p1 = psum.tile([128, TC], F32, tag="p1")   # samples n=0..127
            p2 = psum.tile([128, TC], F32, tag="p2")   # samples n=128..255
            nc.tensor.matmul(p1, a_r[:, 0:128], sa[:, PAD:PAD + TC],
                             start=True, stop=False)
            nc.tensor.matmul(p2, a_r[:, 128:256], sa[:, PAD:PAD + TC],
                             start=True, stop=False)
            nc.tensor.matmul(p1, a_i[:, 0:128], sb[:, PAD:PAD + TC],
                             start=False, stop=True)
            nc.tensor.matmul(p2, a_i[:, 128:256], sb[:, PAD:PAD + TC],
                             start=False, stop=True)
            # ---- overlap-add: shifted partition-group accumulation ----
            # (walrus requires equal SB base partitions for 2-SB-input ops,
            #  so accumulate sequentially with the PSUM operand shifted)
            wn = work.tile([32, TC], F32, tag="wn")
            nc.scalar.copy(wn, p1[0:32, :])
            for j in range(1, 4):
                nc.vector.tensor_add(wn[:, j:TC], wn[:, j:TC],
                                     p1[32 * j:32 * j + 32, 0:TC - j])
            for j in range(4, 8):
                nc.vector.tensor_add(wn[:, j:TC], wn[:, j:TC],
                                     p2[32 * (j - 4):32 * (j - 4) + 32, 0:TC - j])
            nc.vector.tensor_mul(wn, wn, invw)

            if last:
                nc.sync.dma_start(out=aps["out"], in_=wn[:, 0:32])
                break

            # ---- STFT: build hop-shifted frame gather via partition copies ----
            ga = work.tile([128, TS], F32, tag="ga")
            gb = work.tile([128, TS], F32, tag="gb")
            nc.vector.tensor_copy(ga[0:32, :], wn[:, 0:TS])
            nc.scalar.copy(ga[32:64, :], wn[:, 1:1 + TS])
            nc.vector.tensor_copy(ga[64:96, :], wn[:, 2:2 + TS])
            nc.scalar.copy(ga[96:128, :], wn[:, 3:3 + TS])
            t2r = psum.tile([128, TS], F32, tag="t2r")
            t2i = psum.tile([128, TS], F32, tag="t2i")
            nc.tensor.matmul(t2r, bca, ga, start=True, stop=False)
            nc.tensor.matmul(t2i, bia, ga, start=True, stop=False)
            nc.vector.tensor_copy(gb[0:32, :], wn[:, 4:4 + TS])
            nc.scalar.copy(gb[32:64, :], wn[:, 5:5 + TS])
            nc.vector.tensor_copy(gb[64:96, :], wn[:, 6:6 + TS])
            nc.scalar.copy(gb[96:128, :], wn[:, 7:7 + TS])
            nc.tensor.matmul(t2r, bcb, gb, start=False, stop=True)
            nc.tensor.matmul(t2i, bib, gb, start=False, stop=True)

            # ---- phase update: z/|z| carried as (cos, sin) ----
            # ACT runs ONLY Sqrt (one LUT, never switches); rest on DVE.
            rA = work.tile([128, TS], F32, tag="rA")
            iA = work.tile([128, TS], F32, tag="iA")
            nc.vector.tensor_scalar_add(rA, t2r, 1e-6)
            nc.scalar.copy(iA, t2i)
            nc.vector.memset(iA[0:1, :], 0.0)   # Im at DC is exactly 0
            sq = work.tile([128, TS], F32, tag="sq")
            sqi = work.tile([128, TS], F32, tag="sqi")
            nc.vector.tensor_mul(sq, rA, rA)
            nc.vector.tensor_mul(sqi, iA, iA)
            nc.vector.tensor_add(sq, sq, sqi)
            hyp = work.tile([128, TS], F32, tag="hyp")
            nc.scalar.activation(hyp, sq, AF.Sqrt)
            inv = work.tile([128, TS], F32, tag="inv")
            nc.vector.reciprocal(inv, hyp)
            pm = work.tile([128, TS], F32, tag="pm")
            nc.vector.tensor_mul(pm, maga, inv)
            nc.vector.tensor_mul(sa[:, PAD:PAD + TS], rA, pm)
            nc.vector.tensor_mul(sb[:, PAD:PAD + TS], iA, pm)
            # Nyquist row (sb row 0): Im==0 there, so value is mag*sign(Re+eps)
            ge = work.tile([1, TS], F32, tag="ge")
            nc.vector.tensor_scalar(ge, t2i[0:1, :], -1e-6, 2.0,
                                    OP.is_ge, OP.mult)
            nc.vector.scalar_tensor_tensor(sb[0:1, PAD:PAD + TS], ge, 1.0,
                                           magn, OP.subtract, OP.mult)
        if loop is not None:
            loop.__exit__(None, None, None)


_CACHED = None


def _build(rep=1):
    global _CACHED
    if rep == 1 and _CACHED is not None:
        return _CACHED
    nc = bacc.Bacc("TRN2", target_bir_lowering=False, debug=False,
                   num_devices=N_CORES)
    shapes = {
        "a_r": (128, 256), "a_i": (128, 256), "bca": (128, 128),
        "bcb": (128, 128), "bia": (128, 128), "bib": (128, 128),
        "invw": (32, TC), "maga": (128, TS),
        "magn": (1, TS), "sa0": (128, TC + 2 * PAD), "sb0": (128, TC + 2 * PAD),
    }
    aps = {name: nc.dram_tensor(name, shape, F32, kind="ExternalInput").ap()
           for name, shape in shapes.items()}
    aps["out"] = nc.dram_tensor("out", (32, 32), F32, kind="ExternalOutput").ap()
    with tile.TileContext(nc) as t:
        _emit(t, aps, rep=rep)
    nc.compile()
    if rep == 1:
        _CACHED = nc
    return nc


def _host_inputs(mag_b, ph_b):
    """Per-batch host prep: crop, initial cos/sin spec chunks, padding."""
    a_r, a_i, bc, bi, invwsq = _consts()
    mag = np.ascontiguousarray(mag_b[:, :TC]).astype(np.float32)
    ph = np.ascontiguousarray(ph_b[:, :TC]).astype(np.float32)
    sa0 = np.zeros((128, TC + 2 * PAD), np.float32)
    sb0 = np.zeros((128, TC + 2 * PAD), np.float32)
    sa0[:, PAD:PAD + TC] = mag[0:128] * np.cos(ph[0:128])
    sb0[0, PAD:PAD + TC] = mag[128] * np.cos(ph[128])
    sb0[1:, PAD:PAD + TC] = mag[1:128] * np.sin(ph[1:128])
    return {
        "a_r": a_r, "a_i": a_i,
        "bca": np.ascontiguousarray(bc[0:128]), "bcb": np.ascontiguousarray(bc[128:256]),
        "bia": np.ascontiguousarray(bi[0:128]), "bib": np.ascontiguousarray(bi[128:256]),
        "invw": invwsq,
        "maga": np.ascontiguousarray(mag[0:128, :TS]),
        "magn": np.ascontiguousarray(mag[128:129, :TS]),
        "sa0": sa0, "sb0": sb0,
    }


def kernel(mag_spec, phase):
    mag_spec = np.asarray(mag_spec, dtype=np.float32)
    phase = np.asarray(phase, dtype=np.float32)
    nc = _build()
    in_maps = [_host_inputs(mag_spec[c % B], phase[c % B]) for c in range(N_CORES)]
    res = bass_utils.run_bass_kernel_spmd(nc, in_maps, core_ids=list(range(N_CORES)))
    out = np.zeros((B, 1000), np.float32)
    for b in range(B):
        blk = res.results[b]["out"]              # (32, 32): [i, m] = wav[32m+i]
        out[b] = blk.T.reshape(-1)[15:1015]
    return out



# revision 8
# speedup vs baseline: 1.5078x; 1.5078x over previous
"""Griffin-Lim phase reconstruction on Trainium2 (Bass/Tile).

Algorithm identical to the validated baseline (80-frame crop, cos/sin
phase carry; see kernel_baseline.py docstring), but restructured to cut
the per-iteration serial latency:

  * ISTFT + overlap-add fused into 16 accumulating matmuls: tap j of the
    OLA is a K=128 -> M=32 matmul (lhsT = a_r[:, 32j:32j+32]) against a
    column-shifted slice of the spec state.  Taps are spread over the 4
    PE column-groups (tile_position=(0,32g), g=j%4) so 4 matmuls run
    concurrently in the array; the 4 partition-group partial sums are
    folded with 3 DVE adds + the 1/win^2 normalize.  This removes the
    baseline's serial chain of 7 PSUM-source DVE adds.
  * Phase update consolidated: t2 = [Re | Im] halves of ONE PSUM tile, so
    (x+eps)^2 is a single ACT Square over [128, 2*TS] straight from PSUM,
    |z| via ACT Sqrt, 1/|z| via the single-op reciprocal_approx_fast
    (51 ULP) instead of the slow iterative reciprocal, and the spec
    update via two scalar_tensor_tensor ops reading t2 directly.
  * DC/Nyquist rows (sa/sb row 0) via ACT Sign (same LUT set as
    Sqrt/Square/Copy -> no table switches), off the critical path.
  * fp32 everywhere: numerically validated that quantizing the ISTFT
    weights to 16-bit moves the Griffin-Lim fixed point past the error
    budget (host emulation: fp16 weights alone give 2.8e-2 rel err).
"""

import numpy as np
from contextlib import ExitStack

import concourse.bass as bass
import concourse.tile as tile
from concourse import bacc, mybir
from concourse import bass_utils

F32 = mybir.dt.float32
AF = mybir.ActivationFunctionType
OP = mybir.AluOpType

TC = 80           # cropped frame count (of 1000)
TS = TC - 7       # stft / phase-update frame count
PAD = 7
N_ITER = 32
N_FFT = 256
NF = 129
HOP = 32
N_CORES = 8
B = 4


def _consts():
    n = np.arange(N_FFT, dtype=np.float64)
    win = 0.5 - 0.5 * np.cos(2.0 * np.pi * n / N_FFT)
    k = np.arange(128, dtype=np.float64)[:, None]
    ang = 2.0 * np.pi * k * n[None, :] / N_FFT
    ck = np.where(k == 0, 1.0, 2.0) / N_FFT
    a_r = (ck * np.cos(ang) * win[None, :]).astype(np.float32)       # (128,256)
    a_i = (-2.0 / N_FFT * np.sin(ang) * win[None, :]).astype(np.float32)
    a_i[0] = (np.cos(np.pi * n) / N_FFT * win).astype(np.float32)    # Nyquist row
    f = np.arange(128, dtype=np.float64)[None, :]
    ang2 = 2.0 * np.pi * f * n[:, None] / N_FFT                      # (256,128)
    bc = (win[:, None] * np.cos(ang2)).astype(np.float32)
    bi = (-win[:, None] * np.sin(ang2)).astype(np.float32)
    bi[:, 0] = (win * np.cos(np.pi * n)).astype(np.float32)
    L = TC * HOP
    wsq = np.zeros((TC + 8) * HOP + N_FFT, dtype=np.float64)
    w2 = win ** 2
    for t in range(TC + 8):
        wsq[t * HOP:t * HOP + N_FFT] += w2
    wsq = np.maximum(wsq[:L], 1e-8)
    invwsq = (1.0 / wsq).astype(np.float32).reshape(TC, HOP).T.copy()  # (32, TC)
    return a_r, a_i, bc.copy(), bi.copy(), invwsq


def _emit(tc_ctx, aps, rep=1):
    tc = tc_ctx
    nc = tc.nc
    with ExitStack() as ctx:
        consts = ctx.enter_context(tc.tile_pool(name="consts", bufs=1))
        state = ctx.enter_context(tc.tile_pool(name="state", bufs=1))
        work = ctx.enter_context(tc.tile_pool(name="work", bufs=3))
        psum = ctx.enter_context(tc.tile_pool(name="psum", bufs=2, space="PSUM"))

        a_r = consts.tile([128, 256], F32)
        a_i = consts.tile([128, 256], F32)
        bca = consts.tile([128, 128], F32)
        bcb = consts.tile([128, 128], F32)
        bia = consts.tile([128, 128], F32)
        bib = consts.tile([128, 128], F32)
        invw = consts.tile([32, TC], F32)
        maga = consts.tile([128, TS], F32)
        magrow = consts.tile([1, 2 * TS], F32)
        sa = state.tile([128, TC + 2 * PAD], F32)
        sb = state.tile([128, TC + 2 * PAD], F32)
        epsb = consts.tile([128, 1], F32)
        nc.vector.memset(epsb, 1e-6)

        for t, name in [(a_r, "a_r"), (a_i, "a_i"), (bca, "bca"), (bcb, "bcb"),
                        (bia, "bia"), (bib, "bib"),
                        (invw, "invw"), (maga, "maga"), (magrow, "magrow")]:
            nc.sync.dma_start(out=t, in_=aps[name])

        if rep > 1:
            from concourse.engine_type import EngineType
            loop = tc.For_i(0, rep, 1, hint_engines=(
                EngineType.PE, EngineType.DVE, EngineType.Activation,
                EngineType.SP))
        else:
            loop = None
        if loop is not None:
            loop.__enter__()
        nc.sync.dma_start(out=sa, in_=aps["sa0"])
        nc.sync.dma_start(out=sb, in_=aps["sb0"])

        for it in range(N_ITER):
            last = it == N_ITER - 1
            # ---- ISTFT + OLA: 16 col-packed accumulating matmuls ----
            # col-group g accumulates taps {g, g+4} x {sa, sb} into
            # wavps[32g:32g+32]; wav = sum over the 4 partition groups.
            wavps = psum.tile([128, TC], F32, tag="wav")
            for src, w, first in ((sa, a_r, True), (sb, a_i, False)):
                for half in (0, 4):
                    for g in range(4):
                        j = g + half
                        nc.tensor.matmul(
                            wavps[32 * g:32 * g + 32, :],
                            w[:, 32 * j:32 * j + 32],
                            src[:, PAD - j:PAD - j + TC],
                            start=(first and half == 0),
                            stop=((not first) and half == 4),
                            tile_position=(0, 32 * g),
                            skip_group_check=True,
                        )

            if last:
                e1 = work.tile([32, 32], F32, tag="e1")
                e2 = work.tile([32, 32], F32, tag="e2")
                o32 = work.tile([32, 32], F32, tag="o32")
                nc.vector.tensor_copy(e1, wavps[0:32, 0:32])
                nc.scalar.copy(e2, wavps[32:64, 0:32])
                nc.vector.tensor_add(e1, e1, wavps[64:96, 0:32])
                nc.vector.tensor_add(e2, e2, wavps[96:128, 0:32])
                nc.vector.tensor_add(e1, e1, e2)
                nc.vector.tensor_mul(o32, e1, invw[:, 0:32])
                nc.sync.dma_start(out=aps["out"], in_=o32)
                break

            # ---- fold partition groups + window normalize ----
            # (DVE may read only ONE non-scalar PSUM input per op, so the
            #  4-group fold is: 2 parallel copies (DVE+ACT), 2 SBUF+PSUM
            #  adds, combine, normalize)
            q1 = work.tile([32, TC], F32, tag="q1")
            q2 = work.tile([32, TC], F32, tag="q2")
            wn = work.tile([32, TC], F32, tag="wn")
            nc.vector.tensor_copy(q1, wavps[0:32, :])
            nc.scalar.copy(q2, wavps[32:64, :])
            nc.vector.tensor_add(q1, q1, wavps[64:96, :])
            nc.vector.tensor_add(q2, q2, wavps[96:128, :])
            nc.vector.tensor_add(q1, q1, q2)
            nc.vector.tensor_mul(wn, q1, invw)

            # ---- STFT frame gather: 8 hop-shifted partition copies ----
            ga = work.tile([128, TS], F32, tag="ga")
            gb = work.tile([128, TS], F32, tag="gb")
            nc.vector.tensor_copy(ga[0:32, :], wn[:, 0:TS])
            nc.vector.tensor_copy(ga[32:64, :], wn[:, 1:1 + TS])
            nc.vector.tensor_copy(ga[64:96, :], wn[:, 2:2 + TS])
            nc.vector.tensor_copy(ga[96:128, :], wn[:, 3:3 + TS])
            # t2r/t2i must be SEPARATE psum tiles: an accumulation group's
            # start=True clears the whole 2KB zero region of its bank, so
            # two groups can never share a bank at different byte offsets.
            t2r = psum.tile([128, TS], F32, tag="t2r")
            t2i = psum.tile([128, TS], F32, tag="t2i")
            nc.tensor.matmul(t2r, bca, ga, start=True, stop=False)
            nc.tensor.matmul(t2i, bia, ga, start=True, stop=False)
            nc.vector.tensor_copy(gb[0:32, :], wn[:, 4:4 + TS])
            nc.vector.tensor_copy(gb[32:64, :], wn[:, 5:5 + TS])
            nc.scalar.copy(gb[64:96, :], wn[:, 6:6 + TS])
            nc.scalar.copy(gb[96:128, :], wn[:, 7:7 + TS])
            nc.tensor.matmul(t2r, bcb, gb, start=False, stop=True)
            nc.tensor.matmul(t2i, bib, gb, start=False, stop=True)

            # ---- phase update: z/|z| carried as (cos, sin) ----
            # ACT functions used: Square, Sqrt, Sign, Copy -- all in the
            # sqrt_and_others LUT set, so no table switches.
            sgr = work.tile([1, TS], F32, tag="sgr")
            sgi = work.tile([1, TS], F32, tag="sgi")
            nc.scalar.activation(sgr, t2r[0:1, :], AF.Sign, bias=epsb[0:1, :])
            nc.scalar.activation(sgi, t2i[0:1, :], AF.Sign, bias=epsb[0:1, :])
            sq2r = work.tile([128, TS], F32, tag="sq2r")
            sq2i = work.tile([128, TS], F32, tag="sq2i")
            nc.scalar.activation(sq2r, t2r, AF.Square, bias=epsb)
            nc.scalar.activation(sq2i, t2i, AF.Square, bias=epsb)
            sq = work.tile([128, TS], F32, tag="sq")
            nc.vector.tensor_add(sq, sq2r, sq2i)
            hyp = work.tile([128, TS], F32, tag="hyp")
            nc.scalar.activation(hyp, sq, AF.Sqrt)
            inv = work.tile([128, TS], F32, tag="inv")
            nc.vector.reciprocal_approx_fast(out=inv, in_=hyp)
            pm = work.tile([128, TS], F32, tag="pm")
            nc.vector.tensor_mul(pm, maga, inv)
            # sa/sb = (t2 + eps) * pm  (row 0 is overwritten below: PSUM
            # reads must be 32-partition aligned, so compute all 128 rows)
            nc.vector.scalar_tensor_tensor(
                sa[:, PAD:PAD + TS], t2r, 1e-6, pm, OP.add, OP.mult)
            nc.vector.scalar_tensor_tensor(
                sb[:, PAD:PAD + TS], t2i, 1e-6, pm, OP.add, OP.mult)
            # row 0: sa0 = mag_DC * sign(Re_DC+eps); sb0 = mag_Nyq * sign(Re_Nyq+eps)
            nc.vector.tensor_mul(sa[0:1, PAD:PAD + TS], sgr,
                                 magrow[:, 0:TS])
            nc.vector.tensor_mul(sb[0:1, PAD:PAD + TS], sgi,
                                 magrow[:, TS:2 * TS])
        if loop is not None:
            loop.__exit__(None, None, None)


_CACHED = None


def _build(rep=1):
    global _CACHED
    if rep == 1 and _CACHED is not None:
        return _CACHED
    nc = bacc.Bacc("TRN2", target_bir_lowering=False, debug=False,
                   num_devices=N_CORES)
    shapes = {
        "a_r": (128, 256), "a_i": (128, 256), "bca": (128, 128),
        "bcb": (128, 128), "bia": (128, 128), "bib": (128, 128),
        "invw": (32, TC), "maga": (128, TS), "magrow": (1, 2 * TS),
        "sa0": (128, TC + 2 * PAD), "sb0": (128, TC + 2 * PAD),
    }
    aps = {name: nc.dram_tensor(name, shape, F32, kind="ExternalInput").ap()
           for name, shape in shapes.items()}
    aps["out"] = nc.dram_tensor("out", (32, 32), F32, kind="ExternalOutput").ap()
    with tile.TileContext(nc) as t:
        _emit(t, aps, rep=rep)
    nc.compile()
    if rep == 1:
        _CACHED = nc
    return nc


def _host_inputs(mag_b, ph_b):
    """Per-batch host prep: crop, initial cos/sin spec chunks, padding."""
    a_r, a_i, bc, bi, invwsq = _consts()
    mag = np.ascontiguousarray(mag_b[:, :TC]).astype(np.float32)
    ph = np.ascontiguousarray(ph_b[:, :TC]).astype(np.float32)
    sa0 = np.zeros((128, TC + 2 * PAD), np.float32)
    sb0 = np.zeros((128, TC + 2 * PAD), np.float32)
    sa0[:, PAD:PAD + TC] = mag[0:128] * np.cos(ph[0:128])
    sb0[0, PAD:PAD + TC] = mag[128] * np.cos(ph[128])
    sb0[1:, PAD:PAD + TC] = mag[1:128] * np.sin(ph[1:128])
    magrow = np.concatenate([mag[0:1, :TS], mag[128:129, :TS]], axis=1)
    return {
        "a_r": a_r, "a_i": a_i,
        "bca": np.ascontiguousarray(bc[0:128]), "bcb": np.ascontiguousarray(bc[128:256]),
        "bia": np.ascontiguousarray(bi[0:128]), "bib": np.ascontiguousarray(bi[128:256]),
        "invw": invwsq,
        "maga": np.ascontiguousarray(mag[0:128, :TS]),
        "magrow": np.ascontiguousarray(magrow),
        "sa0": sa0, "sb0": sb0,
    }


def kernel(mag_spec, phase):
    mag_spec = np.asarray(mag_spec, dtype=np.float32)
    phase = np.asarray(phase, dtype=np.float32)
    nc = _build()
    in_maps = [_host_inputs(mag_spec[c % B], phase[c % B]) for c in range(N_CORES)]
    res = bass_utils.run_bass_kernel_spmd(nc, in_maps, core_ids=list(range(N_CORES)))
    out = np.zeros((B, 1000), np.float32)
    for b in range(B):
        blk = res.results[b]["out"]              # (32, 32): [i, m] = wav[32m+i]
        out[b] = blk.T.reshape(-1)[15:1015]
    return out
